# revision 7
# baseline (speedup 1.0000x reference)
"""Trainium2 Bass kernel for nn_CustomRNN: 512-step tanh RNN, B=64, H=1024.

  h_t = tanh(W_ih @ x_t + W_hh @ h_{t-1} + b);  out_t = W_ho @ h_t

Strategy (8 NeuronCores, one SPMD program):
  - Time is split into 8 chunks of 64 steps; core i "owns" chunk i.
  - Every core precomputes U = W_ih @ x + b for ITS chunk only (fp32r
    matmuls at full PE rate), then all cores run 8 "rounds". In round r
    every core runs 64 recurrence steps from the same AllReduced h state
    using its local U (only core r's result is meaningful); after the
    round, each core contributes h * mask[r] (mask one-hot on core==r)
    to an AllReduce, which hands the true h to everyone for round r+1.
    Garbage-round hist writes are steered to a trash slot via a per-core
    offset table read into a register.
  - The serial step uses W-stationary fp16 matmuls ([128,128] W_hh^T
    tiles vs h [128,64]); tanh runs on ScalarE with U added on VectorE.
  - Finally each core computes Out = W_ho @ h_hist for its chunk (fp32r)
    and writes its [64, 1024, 64] (t,h,b) output slice.

The fp16 recurrence + fp32r ends give ~1e-3 absmax error on outputs
(error is self-stabilizing: tanh contracts, so fp16 noise saturates).
"""

import numpy as np

B, H, T = 64, 1024, 512
N_CORES = 8
TC = T // N_CORES          # 64 steps per core chunk
KC = H // 128              # 8 contraction chunks
MC = H // 128              # 8 output-row chunks
OCTS = TC // 8             # 8 t-octs in U/Out phases (8 steps x 64 b = 512 cols)

_CACHE = {}


def _build():
    import concourse.bass as bass
    import concourse.mybir as mybir
    import concourse.tile as tile
    from concourse import bacc
    from concourse.bass import ds

    f32 = mybir.dt.float32
    f32r = mybir.dt.float32r
    f16 = mybir.dt.float16

    nc = bacc.Bacc("TRN2", debug=False, num_devices=N_CORES)

    x_chunk = nc.dram_tensor("x_chunk", [TC, H, B], f32r, kind="ExternalInput")
    w_ihT = nc.dram_tensor("w_ihT", [H, H], f32r, kind="ExternalInput")
    w_hhT16 = nc.dram_tensor("w_hhT16", [H, H], f16, kind="ExternalInput")
    w_hoT = nc.dram_tensor("w_hoT", [H, H], f32r, kind="ExternalInput")
    bvec = nc.dram_tensor("bvec", [H, 1], f32, kind="ExternalInput")
    mask8 = nc.dram_tensor("mask8", [1, N_CORES], f32, kind="ExternalInput")
    hoff = nc.dram_tensor("hoff", [1, N_CORES], mybir.dt.int32, kind="ExternalInput")
    out_chunk = nc.dram_tensor("out_chunk", [TC, H, B], f32, kind="ExternalOutput")
    h_n = nc.dram_tensor("h_n", [H, B], f32, kind="ExternalOutput")

    with tile.TileContext(nc) as tc:
        with (
            tc.tile_pool(name="dram_big", bufs=1, space="DRAM") as dram_big,
            tc.tile_pool(name="dram_cc", bufs=4, space="DRAM") as dram_cc,
            tc.tile_pool(name="weights", bufs=1) as weights,
            tc.tile_pool(name="state", bufs=1) as state,
            tc.tile_pool(name="xload", bufs=2) as xload,
            tc.tile_pool(name="uload", bufs=3) as uload,
            tc.tile_pool(name="evict", bufs=3) as evict,
            tc.tile_pool(name="hwork", bufs=3) as hwork,
            tc.tile_pool(name="psum_big", bufs=4, space="PSUM") as psum_big,
            tc.tile_pool(name="psum_st", bufs=4, space="PSUM") as psum_st,
        ):
            U = dram_big.tile([TC, H, B], f32r)
            hist = dram_big.tile([2 * TC, H, B], f32r)

            # ---- load weights / constants into SBUF ----
            wih_sb = weights.tile([128, KC, MC, 128], f32r)
            whh_sb = weights.tile([128, KC, MC, 128], f16)
            who_sb = weights.tile([128, KC, MC, 128], f32r)
            for kc in range(KC):
                for w_sb, w_dram in ((wih_sb, w_ihT), (whh_sb, w_hhT16), (who_sb, w_hoT)):
                    nc.gpsimd.dma_start(
                        out=w_sb[:, kc, :, :],
                        in_=w_dram[kc * 128 : (kc + 1) * 128, :].rearrange(
                            "p (mc m) -> p mc m", m=128
                        ),
                    )
            b_sb = weights.tile([128, MC], f32)
            nc.gpsimd.dma_start(
                out=b_sb, in_=bvec[:, :].rearrange("(mc p) one -> p (mc one)", p=128)
            )
            mask_sb = weights.tile([128, N_CORES], f32)
            nc.gpsimd.dma_start(
                out=mask_sb,
                in_=bass.AP(
                    tensor=mask8.ap().tensor,
                    offset=mask8.ap().offset,
                    ap=[[0, 128], [1, N_CORES]],
                ),
            )
            hoff_sb = weights.tile([1, N_CORES], mybir.dt.int32)
            nc.gpsimd.dma_start(out=hoff_sb, in_=hoff[:, :])

            # ---- phase 1: U = W_ih @ x + b for my chunk ----
            for oct_ in range(OCTS):
                x_sb = xload.tile([128, KC, 8, B], f32r)  # [p, kc, t, b]
                for kc in range(KC):
                    nc.gpsimd.dma_start(
                        out=x_sb[:, kc, :, :],
                        in_=x_chunk[
                            oct_ * 8 : (oct_ + 1) * 8, kc * 128 : (kc + 1) * 128, :
                        ].rearrange("t p b -> p t b"),
                    )
                for mc in range(MC):
                    ps = psum_big.tile([128, 512], f32)
                    for kc in range(KC):
                        nc.tensor.matmul(
                            ps,
                            wih_sb[:, kc, mc, :],
                            x_sb[:, kc, :, :].rearrange("p t b -> p (t b)"),
                            start=(kc == 0),
                            stop=(kc == KC - 1),
                        )
                    u_ev = evict.tile([128, 8, B], f32r)
                    nc.scalar.activation(
                        out=u_ev.rearrange("p t b -> p (t b)"),
                        in_=ps,
                        func=mybir.ActivationFunctionType.Identity,
                        bias=b_sb[:, mc : mc + 1],
                    )
                    nc.gpsimd.dma_start(
                        out=U[oct_ * 8 : (oct_ + 1) * 8, mc * 128 : (mc + 1) * 128, :]
                        .rearrange("t p b -> p t b"),
                        in_=u_ev,
                    )

            # ---- phase 2: 8 rounds of 64 recurrence steps + AllReduce ----
            h16 = [state.tile([128, KC, B], f16, name=f"h16_{i}", tag=f"h16_{i}") for i in range(2)]
            nc.vector.memset(h16[0], 0.0)

            cc_out_last = None
            for rnd in range(N_CORES):
                (_li, (off_val,)) = nc.values_load_multi_w_load_instructions(
                    hoff_sb[0:1, rnd : rnd + 1],
                    engines=[mybir.EngineType.Pool],
                    min_val=0,
                    max_val=TC,
                    skip_runtime_bounds_check=True,
                )

                with tc.For_i(0, TC, 8, hint_engines=(mybir.EngineType.PE,)) as t0:
                    for j in range(8):
                        t = t0 + j
                        hcur = h16[j % 2]
                        hnxt = h16[(j + 1) % 2]
                        u_sb = uload.tile([128, MC, B], f32r)
                        nc.gpsimd.dma_start(
                            out=u_sb,
                            in_=U[ds(t, 1), :, :].rearrange(
                                "s (mc p) b -> p (s mc) b", p=128
                            ),
                        )
                        h32 = hwork.tile([128, MC, B], f32r)
                        for half in range(2):
                            ps = psum_st.tile([128, 4, B], f32)
                            for mi in range(4):
                                mc = half * 4 + mi
                                for kc in range(KC):
                                    nc.tensor.matmul(
                                        ps[:, mi, :],
                                        whh_sb[:, kc, mc, :],
                                        hcur[:, kc, :],
                                        start=(kc == 0),
                                        stop=(kc == KC - 1),
                                    )
                            sl = slice(half * 4, half * 4 + 4)
                            nc.vector.tensor_add(
                                out=h32[:, sl, :],
                                in0=ps,
                                in1=u_sb[:, sl, :],
                            )
                            nc.scalar.activation(
                                out=h32[:, sl, :],
                                in_=h32[:, sl, :],
                                func=mybir.ActivationFunctionType.Tanh,
                            )
                            nc.vector.tensor_copy(out=hnxt[:, sl, :], in_=h32[:, sl, :])
                        nc.gpsimd.dma_start(
                            out=hist[ds(off_val + t, 1), :, :].rearrange(
                                "s (mc p) b -> p (s mc) b", p=128
                            ),
                            in_=h32,
                        )

                # contribute h (one-hot masked) and AllReduce
                cc_in = dram_cc.tile([H, B], f32)
                cc_out = dram_cc.tile([H, B], f32)
                contrib = hwork.tile([128, KC, B], f32)
                nc.vector.tensor_scalar_mul(
                    out=contrib, in0=h16[0], scalar1=mask_sb[:, rnd : rnd + 1]
                )
                nc.gpsimd.dma_start(
                    out=cc_in[:, :].rearrange("(kc p) b -> p kc b", p=128),
                    in_=contrib,
                )
                nc.gpsimd.collective_compute(
                    "AllReduce",
                    mybir.AluOpType.add,
                    replica_groups=[list(range(N_CORES))],
                    ins=[cc_in[:, :].opt()],
                    outs=[cc_out[:, :].opt()],
                )
                h32e = hwork.tile([128, KC, B], f32)
                nc.gpsimd.dma_start(
                    out=h32e, in_=cc_out[:, :].rearrange("(kc p) b -> p kc b", p=128)
                )
                nc.vector.tensor_copy(out=h16[0], in_=h32e)
                cc_out_last = cc_out

            # final hidden state output
            nc.gpsimd.dma_start(out=h_n[:, :], in_=cc_out_last[:, :])

            # ---- phase 3: Out = W_ho @ h_hist for my chunk ----
            for oct_ in range(OCTS):
                hh_sb = xload.tile([128, KC, 8, B], f32r)
                for kc in range(KC):
                    nc.gpsimd.dma_start(
                        out=hh_sb[:, kc, :, :],
                        in_=hist[
                            oct_ * 8 : (oct_ + 1) * 8, kc * 128 : (kc + 1) * 128, :
                        ].rearrange("t p b -> p t b"),
                    )
                for mc in range(MC):
                    ps = psum_big.tile([128, 512], f32)
                    for kc in range(KC):
                        nc.tensor.matmul(
                            ps,
                            who_sb[:, kc, mc, :],
                            hh_sb[:, kc, :, :].rearrange("p t b -> p (t b)"),
                            start=(kc == 0),
                            stop=(kc == KC - 1),
                        )
                    o_ev = evict.tile([128, 8, B], f32)
                    nc.vector.tensor_copy(
                        out=o_ev.rearrange("p t b -> p (t b)"), in_=ps
                    )
                    nc.gpsimd.dma_start(
                        out=out_chunk[
                            oct_ * 8 : (oct_ + 1) * 8, mc * 128 : (mc + 1) * 128, :
                        ].rearrange("t p b -> p t b"),
                        in_=o_ev,
                    )

    nc.finalize()
    return nc


def kernel(input, W_ih, W_hh, W_ho, b):
    from concourse.bass_utils import run_bass_kernel_spmd

    input = np.asarray(input, dtype=np.float32)
    W_ih = np.asarray(W_ih, dtype=np.float32)
    W_hh = np.asarray(W_hh, dtype=np.float32)
    W_ho = np.asarray(W_ho, dtype=np.float32)
    b = np.asarray(b, dtype=np.float32)

    if "nc" not in _CACHE:
        _CACHE["nc"] = _build()
    nc = _CACHE["nc"]

    x_thb = np.ascontiguousarray(input.transpose(2, 1, 0))      # [T, H, B]
    w_ihT = np.ascontiguousarray(W_ih.T)
    w_hoT = np.ascontiguousarray(W_ho.T)
    w_hhT16 = np.ascontiguousarray(W_hh.T.astype(np.float16))
    bvec = np.ascontiguousarray(b.reshape(H, 1))

    in_maps = []
    for i in range(N_CORES):
        mask = np.zeros((1, N_CORES), dtype=np.float32)
        mask[0, i] = 1.0
        off = np.full((1, N_CORES), TC, dtype=np.int32)
        off[0, i] = 0
        in_maps.append(
            {
                "x_chunk": np.ascontiguousarray(x_thb[TC * i : TC * (i + 1)]),
                "w_ihT": w_ihT,
                "w_hhT16": w_hhT16,
                "w_hoT": w_hoT,
                "bvec": bvec,
                "mask8": mask,
                "hoff": off,
            }
        )

    res = run_bass_kernel_spmd(nc, in_maps, core_ids=list(range(N_CORES)))
    if res.exec_time_ns is not None:
        print(f"HW exec time: {res.exec_time_ns} ns")
        if res.per_core_scope_times:
            print(f"scope times: {res.per_core_scope_times}")
    out_thb = np.concatenate([r["out_chunk"] for r in res.results], axis=0)
    output = np.ascontiguousarray(out_thb.transpose(2, 1, 0))   # [B, H, T]
    h_last = res.results[-1]["h_n"]
    return output.astype(np.float32), h_last.astype(np.float32)
